# revision 4
# baseline (speedup 1.0000x reference)
"""Trainium2 Bass kernel for the MAB-style dense transformer block.

Differences vs the baseline kernel.py (same math, same sharding):
  - et (softmax weights) and vs (values) in bf16: halves SBUF so a full
    head of 16 et tiles stages in SBUF, letting PE run logits and AV in
    long uninterrupted bursts (better p-state) while ACT exps behind.
  - vs column layout per head pair: [v_even(64), ones(64), v_odd(64)].
    The even AV uses lhsT window cols 0:128 (v at rows 0:64, denom at
    row 64); the odd AV uses cols 64:192 (denom at row 0, v at rows
    64:128). The channel-major x block assembles directly from the two
    normalize muls -- no SBUF->SBUF DMA, no partition shifts.
  - attention normalize: one reciprocal [2,1024] + K=1 broadcast
    matmuls into a [128,1024] PSUM + one copy + two muls per head pair
    (replaces per-head copy/recip/copy/mul chains).
  - Pool engine takes the copies/adds/squares; DVE keeps muls and the
    stat chain. Persistent ones row (no per-LN DMA).

Sharding: 8 cores = batch (4) x query-half (2), zero cross-core comms.
"""

import os
import sys

sys.path.insert(0, "/opt/trn_rl_repo")

VARIANT = os.environ.get("K2_VARIANT", "full")
PIPE = os.environ.get("K2_PIPE", "lag0")

from contextlib import ExitStack

import numpy as np

import concourse.bass as bass
import concourse.tile as tile
from concourse import bacc, mybir
from concourse.bass_utils import run_bass_kernel_spmd

F32 = mybir.dt.float32
F32R = mybir.dt.float32r
BF16 = mybir.dt.bfloat16

B, D, H, DK = 4, 512, 8, 64
SQ, SK = 2048, 2048
QC = SQ // 2          # per-core query columns
CB = D // 128         # channel blocks of 128
KB = SK // 128        # key blocks of 128
NCH = 512             # matmul moving-dim chunk
SCALE = DK ** -0.5
EPS = 1e-12
VW = 192              # vs cols per head pair: [v_e(64), ones(64), v_o(64)]


def emit_core_kernel(ctx: ExitStack, tc: tile.TileContext, ins: dict, out_ap: bass.AP):
    nc = tc.nc
    EXP = mybir.ActivationFunctionType.Exp
    SQRT = mybir.ActivationFunctionType.Sqrt
    RELU = mybir.ActivationFunctionType.Relu
    ADD = mybir.AluOpType.add
    MULT = mybir.AluOpType.mult

    p_const = ctx.enter_context(tc.tile_pool(name="const", bufs=1))
    p_persist = ctx.enter_context(tc.tile_pool(name="persist", bufs=1))

    # ---- constants ----
    # woT in bf16 (the conv rhs y0 is bf16; matmul inputs must match width);
    # gpsimd-initiated DMAs cast during the transfer
    woT = [p_const.tile([128, D], BF16, tag=f"woT{ci}", name=f"woT{ci}") for ci in range(CB)]
    for ci in range(CB):
        nc.gpsimd.dma_start(out=woT[ci], in_=ins["WoT"][ci * 128:(ci + 1) * 128, :])

    def load_col_vec(name):
        # [512] dram -> [128, CB] sbuf, channel c at (partition c%128, col c//128)
        t = p_const.tile([128, CB], F32, tag=name)
        nc.sync.dma_start(out=t, in_=ins[name].rearrange("(m p) -> p m", p=128))
        return t

    bq_pp = load_col_vec("bq")
    bk_pp = load_col_vec("bk")
    bo_pp = load_col_vec("bo")

    bv_bc = p_const.tile([128, D], F32, tag="bv_bc", name="bv_bc")
    bv_in = ins["bv"]
    nc.sync.dma_start(
        out=bv_bc,
        in_=bass.AP(tensor=bv_in.tensor, offset=bv_in.offset,
                    ap=[[0, 128]] + bv_in.ap),
    )

    def load_gb(gname, bname, tag):
        t = p_const.tile([2, D], F32R, tag=tag)
        nc.sync.dma_start(out=t[0:1, :], in_=ins[gname][None, :])
        nc.sync.dma_start(out=t[1:2, :], in_=ins[bname][None, :])
        return t

    gb0 = load_gb("g0", "b0", "gb0")
    gb1 = load_gb("g1", "b1", "gb1")

    ones_col = p_const.tile([128, 1], F32R, tag="ones_col", name="ones_col")
    nc.sync.dma_start(out=ones_col, in_=ins["ones_c"][:, 0:1])
    ones_col_bf = p_const.tile([128, 1], BF16, tag="ones_col_bf",
                               name="ones_col_bf")
    nc.gpsimd.memset(ones_col_bf, 1.0)
    onesT = p_const.tile([65, DK], F32R, tag="onesT", name="onesT")
    nc.sync.dma_start(out=onesT[64:65, :], in_=ins["ones_q"][None, 0:DK])
    ones_row = p_const.tile([1, DK], F32R, tag="ones_row", name="ones_row")
    nc.sync.dma_start(out=ones_row, in_=ins["ones_q"][None, 0:DK])
    # rhsB rows: [0] = -mean*rstd (per LN), [1] = ones (loaded once)
    rhsB = p_const.tile([2, QC], F32R, tag="rhsB", name="rhsB")
    nc.sync.dma_start(out=rhsB[1:2, :], in_=ins["ones_q"][None, :])
    eps_t = p_const.tile([1, 1], F32, tag="eps", name="eps")
    nc.vector.memset(eps_t, EPS)


    # ---- persistent activations ----
    qch = [p_persist.tile([128, QC], BF16, tag=f"qch{m}", name=f"qch{m}") for m in range(CB)]
    kch = [p_persist.tile([128, SK], BF16, tag=f"kch{m}", name=f"kch{m}") for m in range(CB)]
    # vs[sb]: [128, 4*130] bf16; pair P: cols [130P..130P+64] = [v_even, 1],
    # cols [130P+65..130P+129] = [1, v_odd]
    vs = [p_persist.tile([128, CB * VW], BF16, tag=f"vs{sb}", name=f"vs{sb}")
          for sb in range(KB)]

    # ---- phase 1: projections ----
    # PSUM: ps_big [128,1024] (2 banks) x2; ps_v [128,512] x2 banks
    with tc.tile_pool(name="stage", bufs=1) as p_stage, \
         tc.tile_pool(name="psbigP", bufs=2, space="PSUM") as ps_bigP, \
         tc.tile_pool(name="psvP", bufs=2, space="PSUM") as ps_vP:
        wqT = [p_stage.tile([128, D], F32R, tag=f"wqT{ci}", name=f"wqT{ci}") for ci in range(CB)]
        wkT = [p_stage.tile([128, D], F32R, tag=f"wkT{ci}", name=f"wkT{ci}") for ci in range(CB)]
        wvT = [p_stage.tile([128, D], F32R, tag=f"wvT{ci}", name=f"wvT{ci}") for ci in range(CB)]
        qc = [p_stage.tile([128, QC], F32R, tag=f"qc{ci}", name=f"qc{ci}") for ci in range(CB)]
        kc = [p_stage.tile([128, SK], F32R, tag=f"kc{ci}", name=f"kc{ci}") for ci in range(CB)]
        for ci in range(CB):
            sl = slice(ci * 128, (ci + 1) * 128)
            nc.sync.dma_start(out=wvT[ci], in_=ins["WvT"][sl, :])
            nc.sync.dma_start(out=wqT[ci], in_=ins["WqT"][sl, :])
            nc.sync.dma_start(out=wkT[ci], in_=ins["WkT"][sl, :])
            nc.sync.dma_start(out=qc[ci], in_=ins["Qc"][sl, :])
            nc.sync.dma_start(out=kc[ci], in_=ins["Kc"][sl, :])

        # v projection (sequence-major); write [v,1|1,v] pair layout in bf16
        for sb in range(KB):
            ps = ps_vP.tile([128, NCH], F32, tag="v", name="vps")
            for ci in range(CB):
                nc.tensor.matmul(
                    out=ps,
                    lhsT=(kc[ci][:, sb * 128:(sb + 1) * 128]),
                    rhs=(wvT[ci][:, 0:D]),
                    start=(ci == 0), stop=(ci == CB - 1),
                )
            vt = vs[sb]
            # ones columns: pair P cols [192P+64 .. 192P+127]
            nc.gpsimd.memset(
                bass.AP(tensor=vt.tensor, offset=vt.offset + 64,
                        ap=[vt.ap[0], [VW, CB], [1, DK]]), 1.0)
            # v values: [p, pair(4), parity(2), d(64)]; odd head at +128
            v_out = bass.AP(tensor=vt.tensor, offset=vt.offset,
                            ap=[vt.ap[0], [VW, CB], [128, 2], [1, DK]])
            nc.vector.tensor_add(
                v_out,
                ps.rearrange("p (a b d) -> p a b d", a=CB, b=2),
                bv_bc.rearrange("p (a b d) -> p a b d", a=CB, b=2),
            )

        # q projection: qch[m] = sum_ci WqT[ci]^T-block @ Qc[ci] + bq
        for m in range(CB):
            mcols = slice(m * 128, (m + 1) * 128)
            ps = ps_bigP.tile([128, QC], F32, tag="big", name="qps")
            for n0 in range(0, QC, NCH):
                for ci in range(CB):
                    nc.tensor.matmul(
                        out=ps[:, n0:n0 + NCH],
                        lhsT=(wqT[ci][:, mcols]),
                        rhs=(qc[ci][:, n0:n0 + NCH]),
                        start=(ci == 0), stop=(ci == CB - 1),
                    )
            nc.vector.tensor_scalar(
                out=qch[m], in0=ps, scalar1=bq_pp[:, m:m + 1], scalar2=None, op0=ADD)

        # k projection (channel-major); bias-add alternates DVE / Pool
        for m in range(CB):
            mcols = slice(m * 128, (m + 1) * 128)
            for half in range(2):
                ps = ps_bigP.tile([128, QC], F32, tag="big", name="kps")
                for ci2, n0 in enumerate(range(half * QC, half * QC + QC, NCH)):
                    for ci in range(CB):
                        nc.tensor.matmul(
                            out=ps[:, ci2 * NCH:(ci2 + 1) * NCH],
                            lhsT=(wkT[ci][:, mcols]),
                            rhs=(kc[ci][:, n0:n0 + NCH]),
                            start=(ci == 0), stop=(ci == CB - 1),
                        )
                nc.vector.tensor_scalar(
                    out=kch[m][:, half * QC:(half + 1) * QC], in0=ps,
                    scalar1=bk_pp[:, m:m + 1], scalar2=None, op0=ADD)

    if VARIANT == "proj":
        for m in range(CB):
            nc.gpsimd.dma_start(out=out_ap[m * 128:(m + 1) * 128, :], in_=qch[m])
        return

    # ---- phase 2: attention (software-pipelined: AV lags logits/exp by
    # one head so the in-order PE always has ready work) ----
    # PSUM: ps_log 2x[128,1024] (4 banks), ps_avc 4x[128,512] (4 banks)
    ps_log = ctx.enter_context(tc.tile_pool(name="pslog", bufs=2, space="PSUM"))
    ps_avc = ctx.enter_context(tc.tile_pool(name="psavc", bufs=4, space="PSUM"))
    p_et = ctx.enter_context(tc.tile_pool(name="et", bufs=KB + 1))
    p_rec = ctx.enter_context(tc.tile_pool(name="rec", bufs=1))
    p_rbc = ctx.enter_context(tc.tile_pool(name="rbc", bufs=1))
    p_xz = ctx.enter_context(tc.tile_pool(name="xz", bufs=5))
    p_sq = ctx.enter_context(tc.tile_pool(name="sq", bufs=4))
    p_work = ctx.enter_context(tc.tile_pool(name="work", bufs=4))
    p_tmp = ctx.enter_context(tc.tile_pool(name="tmp", bufs=2))
    p_small = ctx.enter_context(tc.tile_pool(name="small", bufs=4))
    p_acc = ctx.enter_context(tc.tile_pool(name="acc", bufs=1))

    x = [None] * CB

    def emit_logits(h, kb):
        m = h // 2
        hsl = slice((h % 2) * DK, (h % 2) * DK + DK)
        lps = ps_log.tile([128, QC], F32, tag="log", name="lps")
        for n0 in range(0, QC, NCH):
            nc.tensor.matmul(
                out=lps[:, n0:n0 + NCH],
                lhsT=(kch[m][hsl, kb * 128:(kb + 1) * 128]),
                rhs=(qch[m][hsl, n0:n0 + NCH]),
                start=True, stop=True,
            )
        et = p_et.tile([128, QC], BF16, tag="et", name="et")
        if VARIANT == "noexp":
            nc.gpsimd.memset(et[:, 0:1], 0.001)
        else:
            nc.scalar.activation(et, lps, EXP, bias=0.0, scale=SCALE)
        return et

    def emit_av(h, et, kb, avc):
        # even head: window 0:128 -> v rows 0:64, denom row 64
        # odd head: window 64:192 -> denom row 0, v rows 64:128
        m = h // 2
        w0 = m * VW if h % 2 == 0 else m * VW + DK
        lh = vs[kb][:, w0:w0 + 128]
        for c, n0 in enumerate(range(0, QC, NCH)):
            nc.tensor.matmul(
                out=avc[c],
                lhsT=lh,
                rhs=(et[:, n0:n0 + NCH]),
                start=(kb == 0), stop=(kb == KB - 1),
            )

    def emit_pair_epilogue(m, avcE, avcO):
        # denominators: even at PSUM row 64, odd at PSUM row 0
        recsE = p_rec.tile([65, QC], F32R, tag="recsE", name="recsE")
        recsO = p_rec.tile([1, QC], F32R, tag="recsO", name="recsO")
        for c, n0 in enumerate(range(0, QC, NCH)):
            nc.vector.tensor_copy(recsE[64:65, n0:n0 + NCH], avcE[c][64:65, :])
            nc.vector.tensor_copy(recsO[:, n0:n0 + NCH], avcO[c][0:1, :])
        nc.vector.reciprocal(recsE[64:65, :], recsE[64:65, :])
        nc.vector.reciprocal(recsO, recsO)
        # broadcast 1/denom across partitions: bounce through DRAM scratch,
        # read back with a stride-0 partition AP (same pattern as bv_bc)
        rbc = p_rbc.tile([128, QC], F32R, tag="rbc", name="rbc")
        drE, drO = ins[f"scrE{m}"], ins[f"scrO{m}"]
        nc.sync.dma_start(out=drE[None, :], in_=recsE[64:65, :])
        nc.sync.dma_start(out=drO[None, :], in_=recsO[0:1, :])
        nc.sync.dma_start(
            out=rbc[0:DK, :],
            in_=bass.AP(tensor=drE.tensor, offset=drE.offset,
                        ap=[[0, DK]] + drE.ap))
        nc.sync.dma_start(
            out=rbc[DK:128, :],
            in_=bass.AP(tensor=drO.tensor, offset=drO.offset,
                        ap=[[0, DK]] + drO.ap))
        # xatt: even head rows 0:64, odd head rows 64:128, direct
        xatt = p_tmp.tile([128, QC], F32R, tag="tmp", name="xatt")
        for c, n0 in enumerate(range(0, QC, NCH)):
            nc.vector.tensor_mul(xatt[0:DK, n0:n0 + NCH],
                                 avcE[c][0:DK, :], rbc[0:DK, n0:n0 + NCH])
            nc.vector.tensor_mul(xatt[DK:128, n0:n0 + NCH],
                                 avcO[c][DK:128, :], rbc[DK:128, n0:n0 + NCH])
        xm = p_xz.tile([128, QC], F32R, tag="xz", name="xz")
        nc.gpsimd.tensor_add(xm, xatt, qch[m])
        x[m] = xm
        # x^2 for LN0 stats (Pool, off critical path)
        sq_t = p_sq.tile([128, QC], BF16, tag="sq", name="sq")
        nc.gpsimd.tensor_mul(sq_t, xm, xm)
        return sq_t

    def emit_stats_partial(block, sq_t, acc_sum, acc_sq, first):
        # one block's contribution to channel-axis sum/sumsq, accumulated
        # in SBUF by DVE (transient PSUM slots, no long-held banks)
        for c, n0 in enumerate(range(0, QC, NCH)):
            sps = ps_avc.tile([128, NCH], F32, tag="avc", name="sps")
            nc.tensor.matmul(out=sps[0:1, :], lhsT=ones_col,
                             rhs=block[:, n0:n0 + NCH], start=True, stop=True)
            qps = ps_avc.tile([128, NCH], F32, tag="avc", name="qps")
            nc.tensor.matmul(out=qps[0:1, :], lhsT=ones_col_bf,
                             rhs=sq_t[:, n0:n0 + NCH], start=True, stop=True)
            if first:
                nc.vector.tensor_copy(acc_sum[:, n0:n0 + NCH], sps[0:1, :])
                nc.vector.tensor_copy(acc_sq[:, n0:n0 + NCH], qps[0:1, :])
            else:
                nc.vector.tensor_add(acc_sum[:, n0:n0 + NCH],
                                     acc_sum[:, n0:n0 + NCH], sps[0:1, :])
                nc.vector.tensor_add(acc_sq[:, n0:n0 + NCH],
                                     acc_sq[:, n0:n0 + NCH], qps[0:1, :])

    # pipelined head loop: iteration h emits L/exp for head h and AV for
    # head h-1 kb-interleaved; pair epilogues slot in after the odd AV.
    ets_prev = None
    avcs = {}
    sq0 = [None] * CB
    if PIPE == "lag1":
        for h in range(H + 1):
            ets = []
            avc = None
            for kb in range(KB):
                if h < H:
                    ets.append(emit_logits(h, kb))
                if h > 0:
                    if avc is None:
                        avc = [ps_avc.tile([128, NCH], F32, tag="avc",
                                           name=f"avc{h}_{c}")
                               for c in range(QC // NCH)]
                    emit_av(h - 1, ets_prev[kb], kb, avc)
            if h > 0:
                avcs[h - 1] = avc
                if (h - 1) % 2 == 1:
                    m = (h - 1) // 2
                    sq0[m] = emit_pair_epilogue(m, avcs[h - 2], avcs[h - 1])
                    del avcs[h - 2], avcs[h - 1]
            ets_prev = ets
    elif PIPE.startswith("lag") and int(PIPE[3:]) > 0:
        LAG = int(PIPE[3:])  # kb-lag; burst size = LAG
        for h in range(H):
            ets = []
            avc = [ps_avc.tile([128, NCH], F32, tag="avc", name=f"avc{h}_{c}")
                   for c in range(QC // NCH)]
            for b0 in range(0, KB + LAG, LAG):
                for kb in range(b0, min(b0 + LAG, KB)):
                    ets.append(emit_logits(h, kb))
                for kb in range(b0 - LAG, min(b0, KB)):
                    if kb >= 0:
                        emit_av(h, ets[kb], kb, avc)
            avcs[h] = avc
            if h % 2 == 1:
                m = h // 2
                sq0[m] = emit_pair_epilogue(m, avcs[h - 1], avcs[h])
                del avcs[h - 1], avcs[h]
    else:
        acc0 = p_acc.tile([1, 2 * QC], F32, tag="acc0", name="acc0")
        acc_sum0, acc_sq0 = acc0[:, 0:QC], acc0[:, QC:2 * QC]
        for h in range(H):
            ets = [emit_logits(h, kb) for kb in range(KB)]
            avc = [ps_avc.tile([128, NCH], F32, tag="avc", name=f"avc{h}_{c}")
                   for c in range(QC // NCH)]
            for kb in range(KB):
                emit_av(h, ets[kb], kb, avc)
            avcs[h] = avc
            if h % 2 == 1:
                m = h // 2
                sq0[m] = emit_pair_epilogue(m, avcs[h - 1], avcs[h])
                del avcs[h - 1], avcs[h]
                # LN0 partial stats for the PREVIOUS pair: its x/sq are a
                # full head old, so these matmuls never stall the PE
                if m > 0:
                    emit_stats_partial(x[m - 1], sq0[m - 1],
                                       acc_sum0, acc_sq0, m - 1 == 0)

    if VARIANT == "notail":
        for m in range(CB):
            nc.sync.dma_start(out=out_ap[m * 128:(m + 1) * 128, :], in_=x[m])
        return

    # ---- phase 3: tail (LN0 -> conv+relu residual -> LN1) ----
    if PIPE == "lag0":
        emit_stats_partial(x[CB - 1], sq0[CB - 1], acc_sum0, acc_sq0, False)
    else:
        acc0 = p_acc.tile([1, 2 * QC], F32, tag="acc0", name="acc0")
        acc_sum0, acc_sq0 = acc0[:, 0:QC], acc0[:, QC:2 * QC]
        for m in range(CB):
            emit_stats_partial(x[m], sq0[m], acc_sum0, acc_sq0, m == 0)

    def ln_finish(acc_sum, acc_sq, gb, blocks, out_pool, out_tag,
                  out_dtype=F32R):
        # fused stats chain (5 ops):
        #   t1 = (acc_sum/D)*acc_sum = D*mean^2
        #   t2 = acc_sq - t1 = D*var
        #   sd = sqrt(t2/D + eps); rstd = 1/sd
        #   rhsB[0] = (-acc_sum/D)*rstd = -mean*rstd
        t1 = p_small.tile([1, QC], F32, tag="stat", name="t1")
        nc.vector.scalar_tensor_tensor(
            out=t1, in0=acc_sum, scalar=1.0 / D, in1=acc_sum,
            op0=MULT, op1=MULT)
        t2 = p_small.tile([1, QC], F32, tag="stat", name="t2")
        nc.vector.scalar_tensor_tensor(
            out=t2, in0=t1, scalar=-1.0, in1=acc_sq, op0=MULT, op1=ADD)
        sd = p_small.tile([1, QC], F32, tag="stat", name="sd")
        nc.scalar.activation(sd, t2, SQRT, bias=eps_t, scale=1.0 / D)
        rstd = p_small.tile([1, QC], F32R, tag="stat", name="rstd")
        nc.vector.reciprocal(rstd, sd)
        nc.vector.scalar_tensor_tensor(
            out=rhsB[0:1, :], in0=acc_sum, scalar=-1.0 / D, in1=rstd,
            op0=MULT, op1=MULT)

        outs = []
        for m in range(CB):
            mcols = slice(m * 128, (m + 1) * 128)
            a_ps = ps_log.tile([128, QC], F32, tag="log", name="a_ps")
            b_ps = ps_log.tile([128, QC], F32, tag="log", name="b_ps")
            for n0 in range(0, QC, NCH):
                nc.tensor.matmul(
                    out=a_ps[:, n0:n0 + NCH],
                    lhsT=(gb[0:1, mcols]),
                    rhs=(rstd[:, n0:n0 + NCH]),
                    start=True, stop=True,
                )
                nc.tensor.matmul(
                    out=b_ps[:, n0:n0 + NCH],
                    lhsT=(gb[0:2, mcols]),
                    rhs=(rhsB[:, n0:n0 + NCH]),
                    start=True, stop=True,
                )
            t = p_tmp.tile([128, QC], F32R, tag="tmp", name="lnt")
            nc.vector.tensor_mul(t, blocks[m], a_ps)
            o = out_pool.tile([128, QC], out_dtype, tag=out_tag)
            nc.vector.tensor_add(o, t, b_ps)
            outs.append(o)
        return outs

    y0 = ln_finish(acc_sum0, acc_sq0, gb0, x, p_work, "work", out_dtype=BF16)

    acc1 = p_acc.tile([1, 2 * QC], F32, tag="acc1", name="acc1")
    acc_sum1, acc_sq1 = acc1[:, 0:QC], acc1[:, QC:2 * QC]
    z = []
    for o in range(CB):
        ocols = slice(o * 128, (o + 1) * 128)
        cps = ps_log.tile([128, QC], F32, tag="log", name="cps")
        for n0 in range(0, QC, NCH):
            for ci in range(CB):
                nc.tensor.matmul(
                    out=cps[:, n0:n0 + NCH],
                    lhsT=(woT[ci][:, ocols]),
                    rhs=(y0[ci][:, n0:n0 + NCH]),
                    start=(ci == 0), stop=(ci == CB - 1),
                )
        r_t = p_tmp.tile([128, QC], F32R, tag="tmp", name="relu")
        nc.scalar.activation(r_t, cps, RELU, bias=bo_pp[:, o:o + 1], scale=1.0)
        zo = p_xz.tile([128, QC], F32R, tag="xz", name="xz")
        nc.gpsimd.tensor_add(zo, r_t, y0[o])
        z.append(zo)
        # z^2 + LN1 partial stats interleaved with the conv loop
        sq_z = p_sq.tile([128, QC], BF16, tag="sq", name="sqz")
        nc.gpsimd.tensor_mul(sq_z, zo, zo)
        emit_stats_partial(zo, sq_z, acc_sum1, acc_sq1, o == 0)

    fin = ln_finish(acc_sum1, acc_sq1, gb1, z, p_xz, "xz")
    for m in range(CB):
        nc.sync.dma_start(out=out_ap[m * 128:(m + 1) * 128, :], in_=fin[m])


def declare_inputs(nc):
    ins = {}
    for m in range(CB):
        ins[f"scrE{m}"] = nc.dram_tensor(f"scrE{m}", [QC], F32R,
                                         kind="Internal").ap()
        ins[f"scrO{m}"] = nc.dram_tensor(f"scrO{m}", [QC], F32R,
                                         kind="Internal").ap()
    ins["Qc"] = nc.dram_tensor("Qc", [D, QC], F32R, kind="ExternalInput").ap()
    ins["Kc"] = nc.dram_tensor("Kc", [D, SK], F32R, kind="ExternalInput").ap()
    for w in ("WqT", "WkT", "WvT", "WoT"):
        ins[w] = nc.dram_tensor(w, [D, D], F32R, kind="ExternalInput").ap()
    for vname in ("bq", "bk", "bv", "bo"):
        ins[vname] = nc.dram_tensor(vname, [D], F32, kind="ExternalInput").ap()
    for vname in ("g0", "b0", "g1", "b1"):
        ins[vname] = nc.dram_tensor(vname, [D], F32R, kind="ExternalInput").ap()
    ins["ones_c"] = nc.dram_tensor("ones_c", [128, H], F32R,
                                   kind="ExternalInput").ap()
    ins["ones_q"] = nc.dram_tensor("ones_q", [QC], F32R,
                                   kind="ExternalInput").ap()
    return ins


def make_in_maps(inputs):
    Q = np.ascontiguousarray(np.asarray(inputs["Q"], dtype=np.float32))
    K = np.ascontiguousarray(np.asarray(inputs["K"], dtype=np.float32))
    shared = {
        "WqT": np.ascontiguousarray(np.asarray(inputs["Wq"], np.float32).T),
        "WkT": np.ascontiguousarray(np.asarray(inputs["Wk"], np.float32).T),
        "WvT": np.ascontiguousarray(np.asarray(inputs["Wv"], np.float32).T),
        "WoT": np.ascontiguousarray(np.asarray(inputs["Wo"], np.float32).T),
        "bq": np.asarray(inputs["bq"], np.float32),
        "bk": np.asarray(inputs["bk"], np.float32),
        "bv": np.asarray(inputs["bv"], np.float32),
        "bo": np.asarray(inputs["bo"], np.float32),
        "g0": np.asarray(inputs["gamma0"], np.float32),
        "b0": np.asarray(inputs["beta0"], np.float32),
        "g1": np.asarray(inputs["gamma1"], np.float32),
        "b1": np.asarray(inputs["beta1"], np.float32),
        "ones_c": np.ones((128, H), dtype=np.float32),
        "ones_q": np.ones((QC,), dtype=np.float32),
    }
    in_maps = []
    for core in range(8):
        b, j = core // 2, core % 2
        m = dict(shared)
        m["Qc"] = np.ascontiguousarray(Q[b, :, j * QC:(j + 1) * QC])
        m["Kc"] = np.ascontiguousarray(K[b])
        in_maps.append(m)
    return in_maps


def assemble_output(res):
    out = np.empty((B, D, SQ), dtype=np.float32)
    for core in range(8):
        b, j = core // 2, core % 2
        out[b, :, j * QC:(j + 1) * QC] = res.results[core]["out"]
    return out


def build_module():
    nc = bacc.Bacc("TRN2", target_bir_lowering=False, debug=False)
    ins = declare_inputs(nc)
    out_ap = nc.dram_tensor("out", [D, QC], F32R, kind="ExternalOutput").ap()

    with tile.TileContext(nc) as tc:
        with nc.allow_low_precision(reason="f32r/bf16 tiles feed full-rate matmuls"):
            with ExitStack() as ctx:
                emit_core_kernel(ctx, tc, ins, out_ap)
    nc.compile()
    return nc


_NC_CACHE = None


def _get_nc():
    global _NC_CACHE
    if _NC_CACHE is None:
        _NC_CACHE = build_module()
    return _NC_CACHE


def kernel(Q, K, Wq, bq, Wk, bk, Wv, bv, Wo, bo, gamma0, beta0, gamma1, beta1,
           _trace=False, _trace_cores=None):
    in_maps = make_in_maps(dict(
        Q=Q, K=K, Wq=Wq, bq=bq, Wk=Wk, bk=bk, Wv=Wv, bv=bv, Wo=Wo, bo=bo,
        gamma0=gamma0, beta0=beta0, gamma1=gamma1, beta1=beta1))

    nc = _get_nc()
    res = run_bass_kernel_spmd(
        nc, in_maps, core_ids=list(range(8)),
        trace=_trace, trace_cores=_trace_cores,
    )
    out = assemble_output(res)
    if _trace:
        kernel._last_result = res
    return out


# revision 6
# speedup vs baseline: 1.0352x; 1.0352x over previous
"""Trainium2 Bass kernel for the MAB-style dense transformer block.

Structure (per core: batch b = core//2, query-half j = core%2, QC=1024):
  proj (f32r matmuls) -> attention (bf16 qk/et/vs) -> LN0 -> conv+relu
  residual -> LN1, all on one core; zero cross-core communication.

Key measured-on-HW design points (For_i-loop steady-state deltas):
  - qch/kch/et/vs in bf16 (f32r DRAM loads; DVE/ACT cast on write).
    Projection matmuls stay f32r: bf16 proj inputs measured ~40us SLOWER.
  - vs pair layout [v_even(64), ones(64), v_odd(64)] (192 cols): the even
    AV uses lhsT window cols 0:128 (v rows 0:64, denominator row 64); the
    odd AV uses cols 64:192 (denominator row 0, v rows 64:128). Junk rows
    are free (matmul cost = moving columns), and the channel-major x
    block assembles directly from two normalize muls -- no SBUF->SBUF
    DMA, no partition shifts, no illegal PE tile positions.
  - softmax denominators ride along as ones-columns in the AV lhsT; the
    1/denom row is broadcast across partitions by bouncing through a
    DRAM scratch and re-reading with a stride-0 partition AP (PE K=1
    broadcasts to out-base 64 fail the ISA check; SBUF stride-0 DMA
    sources are rejected; Pool cannot read PSUM).
  - plain per-head emission (L*16+exp interleaved, then AV*16) measured
    FASTER than every software-pipelined interleave tried (lag1 head,
    lag2/4/8 kb bursts all regressed: cross-engine sem chatter).
  - LN stats accumulate in SBUF via transient PSUM slots; pairs 0-2 are
    emitted one head late inside the attention stream (their x is old, so
    the PE never stalls); 5-op fused stats chain via scalar_tensor_tensor.
  - Pool pow works as exp on HW but is ~200x too slow (Q7 ucode); ACT
    keeps all 128 exps.

Sharding: 8 cores = batch (4) x query-half (2), zero cross-core comms.
Measured: 458us (session baseline) -> ~390us steady-state For_i loop.
"""

import os
import sys

sys.path.insert(0, "/opt/trn_rl_repo")

VARIANT = os.environ.get("K2_VARIANT", "full")
PIPE = os.environ.get("K2_PIPE", "lag0")

from contextlib import ExitStack

import numpy as np

import concourse.bass as bass
import concourse.tile as tile
from concourse import bacc, mybir
from concourse.bass_utils import run_bass_kernel_spmd

F32 = mybir.dt.float32
F32R = mybir.dt.float32r
BF16 = mybir.dt.bfloat16

B, D, H, DK = 4, 512, 8, 64
SQ, SK = 2048, 2048
QC = SQ // 2          # per-core query columns
CB = D // 128         # channel blocks of 128
KB = SK // 128        # key blocks of 128
NCH = 512             # matmul moving-dim chunk
SCALE = DK ** -0.5
EPS = 1e-12
VW = 192              # vs cols per head pair: [v_e(64), ones(64), v_o(64)]


def emit_core_kernel(ctx: ExitStack, tc: tile.TileContext, ins: dict, out_ap: bass.AP):
    nc = tc.nc
    EXP = mybir.ActivationFunctionType.Exp
    SQRT = mybir.ActivationFunctionType.Sqrt
    RELU = mybir.ActivationFunctionType.Relu
    ADD = mybir.AluOpType.add
    MULT = mybir.AluOpType.mult

    p_const = ctx.enter_context(tc.tile_pool(name="const", bufs=1))
    p_persist = ctx.enter_context(tc.tile_pool(name="persist", bufs=1))

    # ---- constants ----
    # woT in bf16 (the conv rhs y0 is bf16; matmul inputs must match width);
    # gpsimd-initiated DMAs cast during the transfer
    woT = [p_const.tile([128, D], BF16, tag=f"woT{ci}", name=f"woT{ci}") for ci in range(CB)]
    for ci in range(CB):
        nc.gpsimd.dma_start(out=woT[ci], in_=ins["WoT"][ci * 128:(ci + 1) * 128, :])

    def load_col_vec(name):
        # [512] dram -> [128, CB] sbuf, channel c at (partition c%128, col c//128)
        t = p_const.tile([128, CB], F32, tag=name)
        nc.sync.dma_start(out=t, in_=ins[name].rearrange("(m p) -> p m", p=128))
        return t

    bq_pp = load_col_vec("bq")
    bk_pp = load_col_vec("bk")
    bo_pp = load_col_vec("bo")

    bv_bc = p_const.tile([128, D], F32, tag="bv_bc", name="bv_bc")
    bv_in = ins["bv"]
    nc.sync.dma_start(
        out=bv_bc,
        in_=bass.AP(tensor=bv_in.tensor, offset=bv_in.offset,
                    ap=[[0, 128]] + bv_in.ap),
    )

    def load_gb(gname, bname, tag):
        t = p_const.tile([2, D], F32R, tag=tag)
        nc.sync.dma_start(out=t[0:1, :], in_=ins[gname][None, :])
        nc.sync.dma_start(out=t[1:2, :], in_=ins[bname][None, :])
        return t

    gb0 = load_gb("g0", "b0", "gb0")
    gb1 = load_gb("g1", "b1", "gb1")

    ones_col = p_const.tile([128, 1], F32R, tag="ones_col", name="ones_col")
    nc.sync.dma_start(out=ones_col, in_=ins["ones_c"][:, 0:1])
    ones_col_bf = p_const.tile([128, 1], BF16, tag="ones_col_bf",
                               name="ones_col_bf")
    nc.gpsimd.memset(ones_col_bf, 1.0)
    onesT = p_const.tile([65, DK], F32R, tag="onesT", name="onesT")
    nc.sync.dma_start(out=onesT[64:65, :], in_=ins["ones_q"][None, 0:DK])
    ones_row = p_const.tile([1, DK], F32R, tag="ones_row", name="ones_row")
    nc.sync.dma_start(out=ones_row, in_=ins["ones_q"][None, 0:DK])
    # rhsB rows: [0] = -mean*rstd (per LN), [1] = ones (loaded once)
    rhsB = p_const.tile([2, QC], F32R, tag="rhsB", name="rhsB")
    nc.sync.dma_start(out=rhsB[1:2, :], in_=ins["ones_q"][None, :])
    eps_t = p_const.tile([1, 1], F32, tag="eps", name="eps")
    nc.vector.memset(eps_t, EPS)


    # ---- persistent activations ----
    qch = [p_persist.tile([128, QC], BF16, tag=f"qch{m}", name=f"qch{m}") for m in range(CB)]
    kch = [p_persist.tile([128, SK], BF16, tag=f"kch{m}", name=f"kch{m}") for m in range(CB)]
    # vs[sb]: [128, 4*130] bf16; pair P: cols [130P..130P+64] = [v_even, 1],
    # cols [130P+65..130P+129] = [1, v_odd]
    vs = [p_persist.tile([128, CB * VW], BF16, tag=f"vs{sb}", name=f"vs{sb}")
          for sb in range(KB)]

    # ---- phase 1: projections ----
    # PSUM: ps_big [128,1024] (2 banks) x2; ps_v [128,512] x2 banks
    with tc.tile_pool(name="stage", bufs=1) as p_stage, \
         tc.tile_pool(name="psbigP", bufs=2, space="PSUM") as ps_bigP, \
         tc.tile_pool(name="psvP", bufs=2, space="PSUM") as ps_vP:
        wqT = [p_stage.tile([128, D], F32R, tag=f"wqT{ci}", name=f"wqT{ci}") for ci in range(CB)]
        wkT = [p_stage.tile([128, D], F32R, tag=f"wkT{ci}", name=f"wkT{ci}") for ci in range(CB)]
        wvT = [p_stage.tile([128, D], F32R, tag=f"wvT{ci}", name=f"wvT{ci}") for ci in range(CB)]
        qc = [p_stage.tile([128, QC], F32R, tag=f"qc{ci}", name=f"qc{ci}") for ci in range(CB)]
        kc = [p_stage.tile([128, SK], F32R, tag=f"kc{ci}", name=f"kc{ci}") for ci in range(CB)]
        for ci in range(CB):
            sl = slice(ci * 128, (ci + 1) * 128)
            nc.sync.dma_start(out=kc[ci], in_=ins["Kc"][sl, :])
            nc.sync.dma_start(out=wvT[ci], in_=ins["WvT"][sl, :])
        for ci in range(CB):
            sl = slice(ci * 128, (ci + 1) * 128)
            nc.sync.dma_start(out=wqT[ci], in_=ins["WqT"][sl, :])
            nc.sync.dma_start(out=qc[ci], in_=ins["Qc"][sl, :])
            nc.sync.dma_start(out=wkT[ci], in_=ins["WkT"][sl, :])

        # v projection (sequence-major); write [v,1|1,v] pair layout in bf16
        for sb in range(KB):
            ps = ps_vP.tile([128, NCH], F32, tag="v", name="vps")
            for ci in range(CB):
                nc.tensor.matmul(
                    out=ps,
                    lhsT=(kc[ci][:, sb * 128:(sb + 1) * 128]),
                    rhs=(wvT[ci][:, 0:D]),
                    start=(ci == 0), stop=(ci == CB - 1),
                )
            vt = vs[sb]
            # ones columns: pair P cols [192P+64 .. 192P+127]
            nc.gpsimd.memset(
                bass.AP(tensor=vt.tensor, offset=vt.offset + 64,
                        ap=[vt.ap[0], [VW, CB], [1, DK]]), 1.0)
            # v values: [p, pair(4), parity(2), d(64)]; odd head at +128
            v_out = bass.AP(tensor=vt.tensor, offset=vt.offset,
                            ap=[vt.ap[0], [VW, CB], [128, 2], [1, DK]])
            nc.vector.tensor_add(
                v_out,
                ps.rearrange("p (a b d) -> p a b d", a=CB, b=2),
                bv_bc.rearrange("p (a b d) -> p a b d", a=CB, b=2),
            )

        # q projection: qch[m] = sum_ci WqT[ci]^T-block @ Qc[ci] + bq
        for m in range(CB):
            mcols = slice(m * 128, (m + 1) * 128)
            ps = ps_bigP.tile([128, QC], F32, tag="big", name="qps")
            for n0 in range(0, QC, NCH):
                for ci in range(CB):
                    nc.tensor.matmul(
                        out=ps[:, n0:n0 + NCH],
                        lhsT=(wqT[ci][:, mcols]),
                        rhs=(qc[ci][:, n0:n0 + NCH]),
                        start=(ci == 0), stop=(ci == CB - 1),
                    )
            nc.vector.tensor_scalar(
                out=qch[m], in0=ps, scalar1=bq_pp[:, m:m + 1], scalar2=None, op0=ADD)

        # k projection (channel-major); bias-add alternates DVE / Pool
        for m in range(CB):
            mcols = slice(m * 128, (m + 1) * 128)
            for half in range(2):
                ps = ps_bigP.tile([128, QC], F32, tag="big", name="kps")
                for ci2, n0 in enumerate(range(half * QC, half * QC + QC, NCH)):
                    for ci in range(CB):
                        nc.tensor.matmul(
                            out=ps[:, ci2 * NCH:(ci2 + 1) * NCH],
                            lhsT=(wkT[ci][:, mcols]),
                            rhs=(kc[ci][:, n0:n0 + NCH]),
                            start=(ci == 0), stop=(ci == CB - 1),
                        )
                nc.vector.tensor_scalar(
                    out=kch[m][:, half * QC:(half + 1) * QC], in0=ps,
                    scalar1=bk_pp[:, m:m + 1], scalar2=None, op0=ADD)

    if VARIANT == "proj":
        for m in range(CB):
            nc.gpsimd.dma_start(out=out_ap[m * 128:(m + 1) * 128, :], in_=qch[m])
        return

    # ---- phase 2: attention (software-pipelined: AV lags logits/exp by
    # one head so the in-order PE always has ready work) ----
    # PSUM: ps_log 2x[128,1024] (4 banks), ps_avc 4x[128,512] (4 banks)
    ps_log = ctx.enter_context(tc.tile_pool(name="pslog", bufs=2, space="PSUM"))
    ps_avc = ctx.enter_context(tc.tile_pool(name="psavc", bufs=4, space="PSUM"))
    p_et = ctx.enter_context(tc.tile_pool(name="et", bufs=KB + 1))
    p_rec = ctx.enter_context(tc.tile_pool(name="rec", bufs=1))
    p_rbc = ctx.enter_context(tc.tile_pool(name="rbc", bufs=1))
    p_xz = ctx.enter_context(tc.tile_pool(name="xz", bufs=5))
    p_sq = ctx.enter_context(tc.tile_pool(name="sq", bufs=4))
    p_work = ctx.enter_context(tc.tile_pool(name="work", bufs=4))
    p_tmp = ctx.enter_context(tc.tile_pool(name="tmp", bufs=2))
    p_small = ctx.enter_context(tc.tile_pool(name="small", bufs=4))
    p_acc = ctx.enter_context(tc.tile_pool(name="acc", bufs=1))

    x = [None] * CB

    def emit_logits(h, kb):
        m = h // 2
        hsl = slice((h % 2) * DK, (h % 2) * DK + DK)
        lps = ps_log.tile([128, QC], F32, tag="log", name="lps")
        for n0 in range(0, QC, NCH):
            nc.tensor.matmul(
                out=lps[:, n0:n0 + NCH],
                lhsT=(kch[m][hsl, kb * 128:(kb + 1) * 128]),
                rhs=(qch[m][hsl, n0:n0 + NCH]),
                start=True, stop=True,
            )
        et = p_et.tile([128, QC], BF16, tag="et", name="et")
        if VARIANT == "noexp":
            nc.gpsimd.memset(et[:, 0:1], 0.001)
        else:
            nc.scalar.activation(et, lps, EXP, bias=0.0, scale=SCALE)
        return et

    def emit_av(h, et, kb, avc):
        # even head: window 0:128 -> v rows 0:64, denom row 64
        # odd head: window 64:192 -> denom row 0, v rows 64:128
        m = h // 2
        w0 = m * VW if h % 2 == 0 else m * VW + DK
        lh = vs[kb][:, w0:w0 + 128]
        for c, n0 in enumerate(range(0, QC, NCH)):
            nc.tensor.matmul(
                out=avc[c],
                lhsT=lh,
                rhs=(et[:, n0:n0 + NCH]),
                start=(kb == 0), stop=(kb == KB - 1),
            )

    def emit_pair_epilogue(m, avcE, avcO):
        # denominators: even at PSUM row 64, odd at PSUM row 0
        recsE = p_rec.tile([65, QC], F32R, tag="recsE", name="recsE")
        recsO = p_rec.tile([1, QC], F32R, tag="recsO", name="recsO")
        for c, n0 in enumerate(range(0, QC, NCH)):
            nc.vector.tensor_copy(recsE[64:65, n0:n0 + NCH], avcE[c][64:65, :])
            nc.vector.tensor_copy(recsO[:, n0:n0 + NCH], avcO[c][0:1, :])
        nc.vector.reciprocal(recsE[64:65, :], recsE[64:65, :])
        nc.vector.reciprocal(recsO, recsO)
        # broadcast 1/denom across partitions: bounce through DRAM scratch,
        # read back with a stride-0 partition AP (same pattern as bv_bc)
        rbc = p_rbc.tile([128, QC], F32R, tag="rbc", name="rbc")
        drE, drO = ins[f"scrE{m}"], ins[f"scrO{m}"]
        nc.sync.dma_start(out=drE[None, :], in_=recsE[64:65, :])
        nc.sync.dma_start(out=drO[None, :], in_=recsO[0:1, :])
        nc.sync.dma_start(
            out=rbc[0:DK, :],
            in_=bass.AP(tensor=drE.tensor, offset=drE.offset,
                        ap=[[0, DK]] + drE.ap))
        nc.sync.dma_start(
            out=rbc[DK:128, :],
            in_=bass.AP(tensor=drO.tensor, offset=drO.offset,
                        ap=[[0, DK]] + drO.ap))
        # xatt: even head rows 0:64, odd head rows 64:128, direct
        xatt = p_tmp.tile([128, QC], F32R, tag="tmp", name="xatt")
        for c, n0 in enumerate(range(0, QC, NCH)):
            nc.vector.tensor_mul(xatt[0:DK, n0:n0 + NCH],
                                 avcE[c][0:DK, :], rbc[0:DK, n0:n0 + NCH])
            nc.vector.tensor_mul(xatt[DK:128, n0:n0 + NCH],
                                 avcO[c][DK:128, :], rbc[DK:128, n0:n0 + NCH])
        xm = p_xz.tile([128, QC], F32R, tag="xz", name="xz")
        nc.gpsimd.tensor_add(xm, xatt, qch[m])
        x[m] = xm
        # x^2 for LN0 stats (Pool, off critical path)
        sq_t = p_sq.tile([128, QC], BF16, tag="sq", name="sq")
        nc.gpsimd.tensor_mul(sq_t, xm, xm)
        return sq_t

    def emit_stats_partial(block, sq_t, acc_sum, acc_sq, first):
        # one block's contribution to channel-axis sum/sumsq, accumulated
        # in SBUF by DVE (transient PSUM slots, no long-held banks)
        for c, n0 in enumerate(range(0, QC, NCH)):
            sps = ps_avc.tile([128, NCH], F32, tag="avc", name="sps")
            nc.tensor.matmul(out=sps[0:1, :], lhsT=ones_col,
                             rhs=block[:, n0:n0 + NCH], start=True, stop=True)
            qps = ps_avc.tile([128, NCH], F32, tag="avc", name="qps")
            nc.tensor.matmul(out=qps[0:1, :], lhsT=ones_col_bf,
                             rhs=sq_t[:, n0:n0 + NCH], start=True, stop=True)
            if first:
                nc.vector.tensor_copy(acc_sum[:, n0:n0 + NCH], sps[0:1, :])
                nc.vector.tensor_copy(acc_sq[:, n0:n0 + NCH], qps[0:1, :])
            else:
                nc.vector.tensor_add(acc_sum[:, n0:n0 + NCH],
                                     acc_sum[:, n0:n0 + NCH], sps[0:1, :])
                nc.vector.tensor_add(acc_sq[:, n0:n0 + NCH],
                                     acc_sq[:, n0:n0 + NCH], qps[0:1, :])

    # pipelined head loop: iteration h emits L/exp for head h and AV for
    # head h-1 kb-interleaved; pair epilogues slot in after the odd AV.
    ets_prev = None
    avcs = {}
    sq0 = [None] * CB
    if PIPE == "lag1":
        for h in range(H + 1):
            ets = []
            avc = None
            for kb in range(KB):
                if h < H:
                    ets.append(emit_logits(h, kb))
                if h > 0:
                    if avc is None:
                        avc = [ps_avc.tile([128, NCH], F32, tag="avc",
                                           name=f"avc{h}_{c}")
                               for c in range(QC // NCH)]
                    emit_av(h - 1, ets_prev[kb], kb, avc)
            if h > 0:
                avcs[h - 1] = avc
                if (h - 1) % 2 == 1:
                    m = (h - 1) // 2
                    sq0[m] = emit_pair_epilogue(m, avcs[h - 2], avcs[h - 1])
                    del avcs[h - 2], avcs[h - 1]
            ets_prev = ets
    elif PIPE.startswith("lag") and int(PIPE[3:]) > 0:
        LAG = int(PIPE[3:])  # kb-lag; burst size = LAG
        for h in range(H):
            ets = []
            avc = [ps_avc.tile([128, NCH], F32, tag="avc", name=f"avc{h}_{c}")
                   for c in range(QC // NCH)]
            for b0 in range(0, KB + LAG, LAG):
                for kb in range(b0, min(b0 + LAG, KB)):
                    ets.append(emit_logits(h, kb))
                for kb in range(b0 - LAG, min(b0, KB)):
                    if kb >= 0:
                        emit_av(h, ets[kb], kb, avc)
            avcs[h] = avc
            if h % 2 == 1:
                m = h // 2
                sq0[m] = emit_pair_epilogue(m, avcs[h - 1], avcs[h])
                del avcs[h - 1], avcs[h]
    else:
        acc0 = p_acc.tile([1, 2 * QC], F32, tag="acc0", name="acc0")
        acc_sum0, acc_sq0 = acc0[:, 0:QC], acc0[:, QC:2 * QC]
        for h in range(H):
            ets = [emit_logits(h, kb) for kb in range(KB)]
            avc = [ps_avc.tile([128, NCH], F32, tag="avc", name=f"avc{h}_{c}")
                   for c in range(QC // NCH)]
            for kb in range(KB):
                emit_av(h, ets[kb], kb, avc)
            avcs[h] = avc
            if h % 2 == 1:
                m = h // 2
                sq0[m] = emit_pair_epilogue(m, avcs[h - 1], avcs[h])
                del avcs[h - 1], avcs[h]
                # LN0 partial stats for the PREVIOUS pair: its x/sq are a
                # full head old, so these matmuls never stall the PE
                if m > 0:
                    emit_stats_partial(x[m - 1], sq0[m - 1],
                                       acc_sum0, acc_sq0, m - 1 == 0)

    if VARIANT == "notail":
        for m in range(CB):
            nc.sync.dma_start(out=out_ap[m * 128:(m + 1) * 128, :], in_=x[m])
        return

    # ---- phase 3: tail (LN0 -> conv+relu residual -> LN1) ----
    if PIPE == "lag0":
        emit_stats_partial(x[CB - 1], sq0[CB - 1], acc_sum0, acc_sq0, False)
    else:
        acc0 = p_acc.tile([1, 2 * QC], F32, tag="acc0", name="acc0")
        acc_sum0, acc_sq0 = acc0[:, 0:QC], acc0[:, QC:2 * QC]
        for m in range(CB):
            emit_stats_partial(x[m], sq0[m], acc_sum0, acc_sq0, m == 0)

    def ln_finish(acc_sum, acc_sq, gb, blocks, out_pool, out_tag,
                  out_dtype=F32R):
        # fused stats chain (5 ops):
        #   t1 = (acc_sum/D)*acc_sum = D*mean^2
        #   t2 = acc_sq - t1 = D*var
        #   sd = sqrt(t2/D + eps); rstd = 1/sd
        #   rhsB[0] = (-acc_sum/D)*rstd = -mean*rstd
        t1 = p_small.tile([1, QC], F32, tag="stat", name="t1")
        nc.vector.scalar_tensor_tensor(
            out=t1, in0=acc_sum, scalar=1.0 / D, in1=acc_sum,
            op0=MULT, op1=MULT)
        t2 = p_small.tile([1, QC], F32, tag="stat", name="t2")
        nc.vector.scalar_tensor_tensor(
            out=t2, in0=t1, scalar=-1.0, in1=acc_sq, op0=MULT, op1=ADD)
        sd = p_small.tile([1, QC], F32, tag="stat", name="sd")
        nc.scalar.activation(sd, t2, SQRT, bias=eps_t, scale=1.0 / D)
        rstd = p_small.tile([1, QC], F32R, tag="stat", name="rstd")
        nc.vector.reciprocal(rstd, sd)
        nc.vector.scalar_tensor_tensor(
            out=rhsB[0:1, :], in0=acc_sum, scalar=-1.0 / D, in1=rstd,
            op0=MULT, op1=MULT)

        outs = []
        for m in range(CB):
            mcols = slice(m * 128, (m + 1) * 128)
            a_ps = ps_log.tile([128, QC], F32, tag="log", name="a_ps")
            b_ps = ps_log.tile([128, QC], F32, tag="log", name="b_ps")
            for n0 in range(0, QC, NCH):
                nc.tensor.matmul(
                    out=a_ps[:, n0:n0 + NCH],
                    lhsT=(gb[0:1, mcols]),
                    rhs=(rstd[:, n0:n0 + NCH]),
                    start=True, stop=True,
                )
                nc.tensor.matmul(
                    out=b_ps[:, n0:n0 + NCH],
                    lhsT=(gb[0:2, mcols]),
                    rhs=(rhsB[:, n0:n0 + NCH]),
                    start=True, stop=True,
                )
            t = p_tmp.tile([128, QC], F32R, tag="tmp", name="lnt")
            nc.vector.tensor_mul(t, blocks[m], a_ps)
            o = out_pool.tile([128, QC], out_dtype, tag=out_tag)
            nc.vector.tensor_add(o, t, b_ps)
            outs.append(o)
        return outs

    y0 = ln_finish(acc_sum0, acc_sq0, gb0, x, p_work, "work", out_dtype=BF16)

    acc1 = p_acc.tile([1, 2 * QC], F32, tag="acc1", name="acc1")
    acc_sum1, acc_sq1 = acc1[:, 0:QC], acc1[:, QC:2 * QC]
    z = []
    for o in range(CB):
        ocols = slice(o * 128, (o + 1) * 128)
        cps = ps_log.tile([128, QC], F32, tag="log", name="cps")
        for n0 in range(0, QC, NCH):
            for ci in range(CB):
                nc.tensor.matmul(
                    out=cps[:, n0:n0 + NCH],
                    lhsT=(woT[ci][:, ocols]),
                    rhs=(y0[ci][:, n0:n0 + NCH]),
                    start=(ci == 0), stop=(ci == CB - 1),
                )
        r_t = p_tmp.tile([128, QC], F32R, tag="tmp", name="relu")
        nc.scalar.activation(r_t, cps, RELU, bias=bo_pp[:, o:o + 1], scale=1.0)
        zo = p_xz.tile([128, QC], F32R, tag="xz", name="xz")
        nc.gpsimd.tensor_add(zo, r_t, y0[o])
        z.append(zo)
        # z^2 + LN1 partial stats interleaved with the conv loop
        sq_z = p_sq.tile([128, QC], BF16, tag="sq", name="sqz")
        nc.gpsimd.tensor_mul(sq_z, zo, zo)
        emit_stats_partial(zo, sq_z, acc_sum1, acc_sq1, o == 0)

    fin = ln_finish(acc_sum1, acc_sq1, gb1, z, p_xz, "xz")
    for m in range(CB):
        nc.sync.dma_start(out=out_ap[m * 128:(m + 1) * 128, :], in_=fin[m])


def declare_inputs(nc):
    ins = {}
    for m in range(CB):
        ins[f"scrE{m}"] = nc.dram_tensor(f"scrE{m}", [QC], F32R,
                                         kind="Internal").ap()
        ins[f"scrO{m}"] = nc.dram_tensor(f"scrO{m}", [QC], F32R,
                                         kind="Internal").ap()
    ins["Qc"] = nc.dram_tensor("Qc", [D, QC], F32R, kind="ExternalInput").ap()
    ins["Kc"] = nc.dram_tensor("Kc", [D, SK], F32R, kind="ExternalInput").ap()
    for w in ("WqT", "WkT", "WvT", "WoT"):
        ins[w] = nc.dram_tensor(w, [D, D], F32R, kind="ExternalInput").ap()
    for vname in ("bq", "bk", "bv", "bo"):
        ins[vname] = nc.dram_tensor(vname, [D], F32, kind="ExternalInput").ap()
    for vname in ("g0", "b0", "g1", "b1"):
        ins[vname] = nc.dram_tensor(vname, [D], F32R, kind="ExternalInput").ap()
    ins["ones_c"] = nc.dram_tensor("ones_c", [128, H], F32R,
                                   kind="ExternalInput").ap()
    ins["ones_q"] = nc.dram_tensor("ones_q", [QC], F32R,
                                   kind="ExternalInput").ap()
    return ins


def make_in_maps(inputs):
    Q = np.ascontiguousarray(np.asarray(inputs["Q"], dtype=np.float32))
    K = np.ascontiguousarray(np.asarray(inputs["K"], dtype=np.float32))
    shared = {
        "WqT": np.ascontiguousarray(np.asarray(inputs["Wq"], np.float32).T),
        "WkT": np.ascontiguousarray(np.asarray(inputs["Wk"], np.float32).T),
        "WvT": np.ascontiguousarray(np.asarray(inputs["Wv"], np.float32).T),
        "WoT": np.ascontiguousarray(np.asarray(inputs["Wo"], np.float32).T),
        "bq": np.asarray(inputs["bq"], np.float32),
        "bk": np.asarray(inputs["bk"], np.float32),
        "bv": np.asarray(inputs["bv"], np.float32),
        "bo": np.asarray(inputs["bo"], np.float32),
        "g0": np.asarray(inputs["gamma0"], np.float32),
        "b0": np.asarray(inputs["beta0"], np.float32),
        "g1": np.asarray(inputs["gamma1"], np.float32),
        "b1": np.asarray(inputs["beta1"], np.float32),
        "ones_c": np.ones((128, H), dtype=np.float32),
        "ones_q": np.ones((QC,), dtype=np.float32),
    }
    in_maps = []
    for core in range(8):
        b, j = core // 2, core % 2
        m = dict(shared)
        m["Qc"] = np.ascontiguousarray(Q[b, :, j * QC:(j + 1) * QC])
        m["Kc"] = np.ascontiguousarray(K[b])
        in_maps.append(m)
    return in_maps


def assemble_output(res):
    out = np.empty((B, D, SQ), dtype=np.float32)
    for core in range(8):
        b, j = core // 2, core % 2
        out[b, :, j * QC:(j + 1) * QC] = res.results[core]["out"]
    return out


def build_module():
    nc = bacc.Bacc("TRN2", target_bir_lowering=False, debug=False)
    ins = declare_inputs(nc)
    out_ap = nc.dram_tensor("out", [D, QC], F32R, kind="ExternalOutput").ap()

    with tile.TileContext(nc) as tc:
        with nc.allow_low_precision(reason="f32r/bf16 tiles feed full-rate matmuls"):
            with ExitStack() as ctx:
                emit_core_kernel(ctx, tc, ins, out_ap)
    nc.compile()
    return nc


_NC_CACHE = None


def _get_nc():
    global _NC_CACHE
    if _NC_CACHE is None:
        _NC_CACHE = build_module()
    return _NC_CACHE


def kernel(Q, K, Wq, bq, Wk, bk, Wv, bv, Wo, bo, gamma0, beta0, gamma1, beta1,
           _trace=False, _trace_cores=None):
    in_maps = make_in_maps(dict(
        Q=Q, K=K, Wq=Wq, bq=bq, Wk=Wk, bk=bk, Wv=Wv, bv=bv, Wo=Wo, bo=bo,
        gamma0=gamma0, beta0=beta0, gamma1=gamma1, beta1=beta1))

    nc = _get_nc()
    res = run_bass_kernel_spmd(
        nc, in_maps, core_ids=list(range(8)),
        trace=_trace, trace_cores=_trace_cores,
    )
    out = assemble_output(res)
    if _trace:
        kernel._last_result = res
    return out


# revision 7
# speedup vs baseline: 1.0968x; 1.0596x over previous
"""Trainium2 Bass kernel v2 for the MAB-style dense transformer block.

Structure (per core: batch b = core//2, query-half j = core%2, QC=1024):
  proj (f32r matmuls) -> attention (bf16 qk/et/vs) -> LN0 -> conv+relu
  residual -> LN1, all on one core; zero cross-core communication.

Key measured-on-HW design points (For_i-loop steady-state deltas):
  - qch/kch/et/vs in bf16 (f32r DRAM loads; DVE/ACT cast on write).
    Projection matmuls stay f32r: bf16 proj inputs measured ~40us SLOWER.
  - vs pair layout [v_even(64), ones(64), v_odd(64)] (192 cols): the even
    AV uses lhsT window cols 0:128 (v rows 0:64, denominator row 64); the
    odd AV uses cols 64:192 (denominator row 0, v rows 64:128). Junk rows
    are free (matmul cost = moving columns), and the channel-major x
    block assembles directly from two normalize muls -- no SBUF->SBUF
    DMA, no partition shifts, no illegal PE tile positions.
  - softmax denominators ride along as ones-columns in the AV lhsT; the
    1/denom row is broadcast across partitions by bouncing through a
    DRAM scratch and re-reading with a stride-0 partition AP (PE K=1
    broadcasts to out-base 64 fail the ISA check; SBUF stride-0 DMA
    sources are rejected; Pool cannot read PSUM).
  - plain per-head emission (L*16+exp interleaved, then AV*16) measured
    FASTER than every software-pipelined interleave tried (lag1 head,
    lag2/4/8 kb bursts all regressed: cross-engine sem chatter).
  - LN stats accumulate in SBUF via transient PSUM slots; pairs 0-2 are
    emitted one head late inside the attention stream (their x is old, so
    the PE never stalls); 5-op fused stats chain via scalar_tensor_tensor.
  - Pool pow works as exp on HW but is ~200x too slow (Q7 ucode); ACT
    keeps all 128 exps.

Sharding: 8 cores = batch (4) x query-half (2), zero cross-core comms.
Measured: 458us (session baseline) -> ~390us steady-state For_i loop.
"""

import os
import sys

sys.path.insert(0, "/opt/trn_rl_repo")

VARIANT = os.environ.get("K2_VARIANT", "full")
PIPE = os.environ.get("K2_PIPE", "lag0")

from contextlib import ExitStack

import numpy as np

import concourse.bass as bass
import concourse.tile as tile
from concourse import bacc, mybir
from concourse.bass_utils import run_bass_kernel_spmd

F32 = mybir.dt.float32
F32R = mybir.dt.float32r
BF16 = mybir.dt.bfloat16

B, D, H, DK = 4, 512, 8, 64
SQ, SK = 2048, 2048
QC = SQ // 2          # per-core query columns
CB = D // 128         # channel blocks of 128
KB = SK // 128        # key blocks of 128
NCH = 512             # matmul moving-dim chunk
SCALE = DK ** -0.5
EPS = 1e-12
VW = 192              # vs cols per head pair: [v_e(64), ones(64), v_o(64)]


def emit_core_kernel(ctx: ExitStack, tc: tile.TileContext, ins: dict, out_ap: bass.AP):
    nc = tc.nc
    EXP = mybir.ActivationFunctionType.Exp
    SQRT = mybir.ActivationFunctionType.Sqrt
    RELU = mybir.ActivationFunctionType.Relu
    ADD = mybir.AluOpType.add
    MULT = mybir.AluOpType.mult

    p_const = ctx.enter_context(tc.tile_pool(name="const", bufs=1))
    p_persist = ctx.enter_context(tc.tile_pool(name="persist", bufs=1))

    # ---- constants ----
    # woT in bf16 (the conv rhs y0 is bf16; matmul inputs must match width);
    # gpsimd-initiated DMAs cast during the transfer
    woT = [p_const.tile([128, D], BF16, tag=f"woT{ci}", name=f"woT{ci}") for ci in range(CB)]
    for ci in range(CB):
        nc.gpsimd.dma_start(out=woT[ci], in_=ins["WoT"][ci * 128:(ci + 1) * 128, :])

    def load_col_vec(name):
        # [512] dram -> [128, CB] sbuf, channel c at (partition c%128, col c//128)
        t = p_const.tile([128, CB], F32, tag=name)
        nc.sync.dma_start(out=t, in_=ins[name].rearrange("(m p) -> p m", p=128))
        return t

    bq_pp = load_col_vec("bq")
    bk_pp = load_col_vec("bk")
    bo_pp = load_col_vec("bo")

    bv_bc = p_const.tile([128, D], F32, tag="bv_bc", name="bv_bc")
    bv_in = ins["bv"]
    nc.sync.dma_start(
        out=bv_bc,
        in_=bass.AP(tensor=bv_in.tensor, offset=bv_in.offset,
                    ap=[[0, 128]] + bv_in.ap),
    )

    def load_gb(gname, bname, tag):
        t = p_const.tile([2, D], F32R, tag=tag)
        nc.sync.dma_start(out=t[0:1, :], in_=ins[gname][None, :])
        nc.sync.dma_start(out=t[1:2, :], in_=ins[bname][None, :])
        return t

    gb0 = load_gb("g0", "b0", "gb0")
    gb1 = load_gb("g1", "b1", "gb1")

    ones_col = p_const.tile([128, 1], F32R, tag="ones_col", name="ones_col")
    nc.sync.dma_start(out=ones_col, in_=ins["ones_c"][:, 0:1])
    ones_col_bf = p_const.tile([128, 1], BF16, tag="ones_col_bf",
                               name="ones_col_bf")
    nc.gpsimd.memset(ones_col_bf, 1.0)
    onesT = p_const.tile([65, DK], F32R, tag="onesT", name="onesT")
    nc.sync.dma_start(out=onesT[64:65, :], in_=ins["ones_q"][None, 0:DK])
    ones_row = p_const.tile([1, DK], F32R, tag="ones_row", name="ones_row")
    nc.sync.dma_start(out=ones_row, in_=ins["ones_q"][None, 0:DK])
    # rhsB rows: [0] = -mean*rstd (per LN), [1] = ones (loaded once)
    rhsB = p_const.tile([2, QC], F32R, tag="rhsB", name="rhsB")
    nc.sync.dma_start(out=rhsB[1:2, :], in_=ins["ones_q"][None, :])
    eps_t = p_const.tile([1, 1], F32, tag="eps", name="eps")
    nc.vector.memset(eps_t, EPS)


    # ---- persistent activations ----
    qch = [p_persist.tile([128, QC], BF16, tag=f"qch{m}", name=f"qch{m}") for m in range(CB)]
    kch = [p_persist.tile([128, SK], BF16, tag=f"kch{m}", name=f"kch{m}") for m in range(CB)]
    # vs[sb]: [128, 4*130] bf16; pair P: cols [130P..130P+64] = [v_even, 1],
    # cols [130P+65..130P+129] = [1, v_odd]
    vs = [p_persist.tile([128, CB * VW], BF16, tag=f"vs{sb}", name=f"vs{sb}")
          for sb in range(KB)]

    # ---- phase 1: projections ----
    # PSUM: ps_big [128,1024] (2 banks) x2; ps_v [128,512] x2 banks
    with tc.tile_pool(name="stage", bufs=1) as p_stage, \
         tc.tile_pool(name="psbigP", bufs=2, space="PSUM") as ps_bigP, \
         tc.tile_pool(name="psvP", bufs=2, space="PSUM") as ps_vP:
        wqT = [p_stage.tile([128, D], F32R, tag=f"wqT{ci}", name=f"wqT{ci}") for ci in range(CB)]
        wkT = [p_stage.tile([128, D], F32R, tag=f"wkT{ci}", name=f"wkT{ci}") for ci in range(CB)]
        wvT = [p_stage.tile([128, D], F32R, tag=f"wvT{ci}", name=f"wvT{ci}") for ci in range(CB)]
        qc = [p_stage.tile([128, QC], F32R, tag=f"qc{ci}", name=f"qc{ci}") for ci in range(CB)]
        kc = [p_stage.tile([128, SK], F32R, tag=f"kc{ci}", name=f"kc{ci}") for ci in range(CB)]
        for ci in range(CB):
            sl = slice(ci * 128, (ci + 1) * 128)
            nc.sync.dma_start(out=kc[ci], in_=ins["Kc"][sl, :])
            nc.sync.dma_start(out=wvT[ci], in_=ins["WvT"][sl, :])
        for ci in range(CB):
            sl = slice(ci * 128, (ci + 1) * 128)
            nc.sync.dma_start(out=wqT[ci], in_=ins["WqT"][sl, :])
            nc.sync.dma_start(out=qc[ci], in_=ins["Qc"][sl, :])
            nc.sync.dma_start(out=wkT[ci], in_=ins["WkT"][sl, :])

        # v projection (sequence-major); write [v,1|1,v] pair layout in bf16
        for sb in range(KB):
            ps = ps_vP.tile([128, NCH], F32, tag="v", name="vps")
            for ci in range(CB):
                nc.tensor.matmul(
                    out=ps,
                    lhsT=(kc[ci][:, sb * 128:(sb + 1) * 128]),
                    rhs=(wvT[ci][:, 0:D]),
                    start=(ci == 0), stop=(ci == CB - 1),
                )
            vt = vs[sb]
            # ones columns: pair P cols [192P+64 .. 192P+127]
            nc.gpsimd.memset(
                bass.AP(tensor=vt.tensor, offset=vt.offset + 64,
                        ap=[vt.ap[0], [VW, CB], [1, DK]]), 1.0)
            # v values: [p, pair(4), parity(2), d(64)]; odd head at +128
            v_out = bass.AP(tensor=vt.tensor, offset=vt.offset,
                            ap=[vt.ap[0], [VW, CB], [128, 2], [1, DK]])
            nc.vector.tensor_add(
                v_out,
                ps.rearrange("p (a b d) -> p a b d", a=CB, b=2),
                bv_bc.rearrange("p (a b d) -> p a b d", a=CB, b=2),
            )

        # q projection: qch[m] = sum_ci WqT[ci]^T-block @ Qc[ci] + bq
        for m in range(CB):
            mcols = slice(m * 128, (m + 1) * 128)
            ps = ps_bigP.tile([128, QC], F32, tag="big", name="qps")
            for n0 in range(0, QC, NCH):
                for ci in range(CB):
                    nc.tensor.matmul(
                        out=ps[:, n0:n0 + NCH],
                        lhsT=(wqT[ci][:, mcols]),
                        rhs=(qc[ci][:, n0:n0 + NCH]),
                        start=(ci == 0), stop=(ci == CB - 1),
                    )
            nc.scalar.activation(qch[m], ps,
                                 mybir.ActivationFunctionType.Identity,
                                 bias=bq_pp[:, m:m + 1], scale=1.0)

        # k projection (channel-major); bias-add alternates DVE / Pool
        for m in range(CB):
            mcols = slice(m * 128, (m + 1) * 128)
            for half in range(2):
                ps = ps_bigP.tile([128, QC], F32, tag="big", name="kps")
                for ci2, n0 in enumerate(range(half * QC, half * QC + QC, NCH)):
                    for ci in range(CB):
                        nc.tensor.matmul(
                            out=ps[:, ci2 * NCH:(ci2 + 1) * NCH],
                            lhsT=(wkT[ci][:, mcols]),
                            rhs=(kc[ci][:, n0:n0 + NCH]),
                            start=(ci == 0), stop=(ci == CB - 1),
                        )
                nc.scalar.activation(kch[m][:, half * QC:(half + 1) * QC],
                                     ps,
                                     mybir.ActivationFunctionType.Identity,
                                     bias=bk_pp[:, m:m + 1], scale=1.0)

    if VARIANT == "proj":
        for m in range(CB):
            nc.gpsimd.dma_start(out=out_ap[m * 128:(m + 1) * 128, :], in_=qch[m])
        return

    # ---- phase 2: attention (software-pipelined: AV lags logits/exp by
    # one head so the in-order PE always has ready work) ----
    # PSUM: ps_log 2x[128,1024] (4 banks), ps_avc 4x[128,512] (4 banks)
    ps_log = ctx.enter_context(tc.tile_pool(name="pslog", bufs=2, space="PSUM"))
    ps_avc = ctx.enter_context(tc.tile_pool(name="psavc", bufs=4, space="PSUM"))
    p_et = ctx.enter_context(tc.tile_pool(name="et", bufs=KB + 2))
    p_rec = ctx.enter_context(tc.tile_pool(name="rec", bufs=1))
    p_rbc = ctx.enter_context(tc.tile_pool(name="rbc", bufs=1))
    p_xz = ctx.enter_context(tc.tile_pool(name="xz", bufs=5))
    p_sq = ctx.enter_context(tc.tile_pool(name="sq", bufs=4))
    p_work = ctx.enter_context(tc.tile_pool(name="work", bufs=4))
    p_tmp = ctx.enter_context(tc.tile_pool(name="tmp", bufs=2))
    p_small = ctx.enter_context(tc.tile_pool(name="small", bufs=4))
    p_acc = ctx.enter_context(tc.tile_pool(name="acc", bufs=1))

    x = [None] * CB

    def emit_logits(h, kb):
        m = h // 2
        hsl = slice((h % 2) * DK, (h % 2) * DK + DK)
        lps = ps_log.tile([128, QC], F32, tag="log", name="lps")
        for n0 in range(0, QC, NCH):
            nc.tensor.matmul(
                out=lps[:, n0:n0 + NCH],
                lhsT=(kch[m][hsl, kb * 128:(kb + 1) * 128]),
                rhs=(qch[m][hsl, n0:n0 + NCH]),
                start=True, stop=True,
            )
        et = p_et.tile([128, QC], BF16, tag="et", name="et")
        if VARIANT == "noexp":
            nc.gpsimd.memset(et[:, 0:1], 0.001)
        else:
            nc.scalar.activation(et, lps, EXP, bias=0.0, scale=SCALE)
        return et

    def emit_av(h, et, kb, avc):
        # even head: window 0:128 -> v rows 0:64, denom row 64
        # odd head: window 64:192 -> denom row 0, v rows 64:128
        m = h // 2
        w0 = m * VW if h % 2 == 0 else m * VW + DK
        lh = vs[kb][:, w0:w0 + 128]
        for c, n0 in enumerate(range(0, QC, NCH)):
            nc.tensor.matmul(
                out=avc[c],
                lhsT=lh,
                rhs=(et[:, n0:n0 + NCH]),
                start=(kb == 0), stop=(kb == KB - 1),
            )

    def emit_pair_epilogue(m, avcE, avcO):
        # denominators: even at PSUM row 64, odd at PSUM row 0
        recsE = p_rec.tile([65, QC], F32R, tag="recsE", name="recsE")
        recsO = p_rec.tile([1, QC], F32R, tag="recsO", name="recsO")
        for c, n0 in enumerate(range(0, QC, NCH)):
            nc.vector.tensor_copy(recsE[64:65, n0:n0 + NCH], avcE[c][64:65, :])
            nc.vector.tensor_copy(recsO[:, n0:n0 + NCH], avcO[c][0:1, :])
        nc.vector.reciprocal(recsE[64:65, :], recsE[64:65, :])
        nc.vector.reciprocal(recsO, recsO)
        # broadcast 1/denom across partitions: bounce through DRAM scratch,
        # read back with a stride-0 partition AP (same pattern as bv_bc)
        rbc = p_rbc.tile([128, QC], F32R, tag="rbc", name="rbc")
        drE, drO = ins[f"scrE{m}"], ins[f"scrO{m}"]
        nc.sync.dma_start(out=drE[None, :], in_=recsE[64:65, :])
        nc.sync.dma_start(out=drO[None, :], in_=recsO[0:1, :])
        nc.sync.dma_start(
            out=rbc[0:DK, :],
            in_=bass.AP(tensor=drE.tensor, offset=drE.offset,
                        ap=[[0, DK]] + drE.ap))
        nc.sync.dma_start(
            out=rbc[DK:128, :],
            in_=bass.AP(tensor=drO.tensor, offset=drO.offset,
                        ap=[[0, DK]] + drO.ap))
        # xatt: even head rows 0:64, odd head rows 64:128, direct
        xatt = p_tmp.tile([128, QC], F32R, tag="tmp", name="xatt")
        for c, n0 in enumerate(range(0, QC, NCH)):
            nc.vector.tensor_mul(xatt[0:DK, n0:n0 + NCH],
                                 avcE[c][0:DK, :], rbc[0:DK, n0:n0 + NCH])
            nc.vector.tensor_mul(xatt[DK:128, n0:n0 + NCH],
                                 avcO[c][DK:128, :], rbc[DK:128, n0:n0 + NCH])
        xm = p_xz.tile([128, QC], F32R, tag="xz", name="xz")
        nc.gpsimd.tensor_add(xm, xatt, qch[m])
        x[m] = xm
        # x^2 for LN0 stats (Pool, off critical path)
        sq_t = p_sq.tile([128, QC], BF16, tag="sq", name="sq")
        nc.gpsimd.tensor_mul(sq_t, xm, xm)
        return sq_t

    def emit_stats_partial(block, sq_t, acc_sum, acc_sq, first):
        # one block's contribution to channel-axis sum/sumsq, accumulated
        # in SBUF by DVE (transient PSUM slots, no long-held banks)
        for c, n0 in enumerate(range(0, QC, NCH)):
            sps = ps_avc.tile([128, NCH], F32, tag="avc", name="sps")
            nc.tensor.matmul(out=sps[0:1, :], lhsT=ones_col,
                             rhs=block[:, n0:n0 + NCH], start=True, stop=True)
            qps = ps_avc.tile([128, NCH], F32, tag="avc", name="qps")
            nc.tensor.matmul(out=qps[0:1, :], lhsT=ones_col_bf,
                             rhs=sq_t[:, n0:n0 + NCH], start=True, stop=True)
            if first:
                nc.vector.tensor_copy(acc_sum[:, n0:n0 + NCH], sps[0:1, :])
                nc.vector.tensor_copy(acc_sq[:, n0:n0 + NCH], qps[0:1, :])
            else:
                nc.vector.tensor_add(acc_sum[:, n0:n0 + NCH],
                                     acc_sum[:, n0:n0 + NCH], sps[0:1, :])
                nc.vector.tensor_add(acc_sq[:, n0:n0 + NCH],
                                     acc_sq[:, n0:n0 + NCH], qps[0:1, :])

    # pipelined head loop: iteration h emits L/exp for head h and AV for
    # head h-1 kb-interleaved; pair epilogues slot in after the odd AV.
    ets_prev = None
    avcs = {}
    sq0 = [None] * CB
    if PIPE == "lag1":
        for h in range(H + 1):
            ets = []
            avc = None
            for kb in range(KB):
                if h < H:
                    ets.append(emit_logits(h, kb))
                if h > 0:
                    if avc is None:
                        avc = [ps_avc.tile([128, NCH], F32, tag="avc",
                                           name=f"avc{h}_{c}")
                               for c in range(QC // NCH)]
                    emit_av(h - 1, ets_prev[kb], kb, avc)
            if h > 0:
                avcs[h - 1] = avc
                if (h - 1) % 2 == 1:
                    m = (h - 1) // 2
                    sq0[m] = emit_pair_epilogue(m, avcs[h - 2], avcs[h - 1])
                    del avcs[h - 2], avcs[h - 1]
            ets_prev = ets
    elif PIPE.startswith("lag") and int(PIPE[3:]) > 0:
        LAG = int(PIPE[3:])  # kb-lag; burst size = LAG
        for h in range(H):
            ets = []
            avc = [ps_avc.tile([128, NCH], F32, tag="avc", name=f"avc{h}_{c}")
                   for c in range(QC // NCH)]
            for b0 in range(0, KB + LAG, LAG):
                for kb in range(b0, min(b0 + LAG, KB)):
                    ets.append(emit_logits(h, kb))
                for kb in range(b0 - LAG, min(b0, KB)):
                    if kb >= 0:
                        emit_av(h, ets[kb], kb, avc)
            avcs[h] = avc
            if h % 2 == 1:
                m = h // 2
                sq0[m] = emit_pair_epilogue(m, avcs[h - 1], avcs[h])
                del avcs[h - 1], avcs[h]
    else:
        acc0 = p_acc.tile([1, 2 * QC], F32, tag="acc0", name="acc0")
        acc_sum0, acc_sq0 = acc0[:, 0:QC], acc0[:, QC:2 * QC]
        for h in range(H):
            ets = [emit_logits(h, kb) for kb in range(KB)]
            avc = [ps_avc.tile([128, NCH], F32, tag="avc", name=f"avc{h}_{c}")
                   for c in range(QC // NCH)]
            for kb in range(KB):
                emit_av(h, ets[kb], kb, avc)
            avcs[h] = avc
            if h % 2 == 1:
                m = h // 2
                sq0[m] = emit_pair_epilogue(m, avcs[h - 1], avcs[h])
                del avcs[h - 1], avcs[h]
                # LN0 partial stats for the PREVIOUS pair: its x/sq are a
                # full head old, so these matmuls never stall the PE
                if m > 0:
                    emit_stats_partial(x[m - 1], sq0[m - 1],
                                       acc_sum0, acc_sq0, m - 1 == 0)

    if VARIANT == "notail":
        for m in range(CB):
            nc.sync.dma_start(out=out_ap[m * 128:(m + 1) * 128, :], in_=x[m])
        return

    # ---- phase 3: tail (LN0 -> conv+relu residual -> LN1) ----
    if PIPE == "lag0":
        emit_stats_partial(x[CB - 1], sq0[CB - 1], acc_sum0, acc_sq0, False)
    else:
        acc0 = p_acc.tile([1, 2 * QC], F32, tag="acc0", name="acc0")
        acc_sum0, acc_sq0 = acc0[:, 0:QC], acc0[:, QC:2 * QC]
        for m in range(CB):
            emit_stats_partial(x[m], sq0[m], acc_sum0, acc_sq0, m == 0)

    def ln_finish(acc_sum, acc_sq, gb, blocks, out_pool, out_tag,
                  out_dtype=F32R):
        # fused stats chain (5 ops):
        #   t1 = (acc_sum/D)*acc_sum = D*mean^2
        #   t2 = acc_sq - t1 = D*var
        #   sd = sqrt(t2/D + eps); rstd = 1/sd
        #   rhsB[0] = (-acc_sum/D)*rstd = -mean*rstd
        t1 = p_small.tile([1, QC], F32, tag="stat", name="t1")
        nc.vector.scalar_tensor_tensor(
            out=t1, in0=acc_sum, scalar=1.0 / D, in1=acc_sum,
            op0=MULT, op1=MULT)
        t2 = p_small.tile([1, QC], F32, tag="stat", name="t2")
        nc.vector.scalar_tensor_tensor(
            out=t2, in0=t1, scalar=-1.0, in1=acc_sq, op0=MULT, op1=ADD)
        sd = p_small.tile([1, QC], F32, tag="stat", name="sd")
        nc.scalar.activation(sd, t2, SQRT, bias=eps_t, scale=1.0 / D)
        rstd = p_small.tile([1, QC], F32R, tag="stat", name="rstd")
        nc.vector.reciprocal(rstd, sd)
        nc.vector.scalar_tensor_tensor(
            out=rhsB[0:1, :], in0=acc_sum, scalar=-1.0 / D, in1=rstd,
            op0=MULT, op1=MULT)

        outs = []
        for m in range(CB):
            mcols = slice(m * 128, (m + 1) * 128)
            a_ps = ps_log.tile([128, QC], F32, tag="log", name="a_ps")
            b_ps = ps_log.tile([128, QC], F32, tag="log", name="b_ps")
            for n0 in range(0, QC, NCH):
                nc.tensor.matmul(
                    out=a_ps[:, n0:n0 + NCH],
                    lhsT=(gb[0:1, mcols]),
                    rhs=(rstd[:, n0:n0 + NCH]),
                    start=True, stop=True,
                )
                nc.tensor.matmul(
                    out=b_ps[:, n0:n0 + NCH],
                    lhsT=(gb[0:2, mcols]),
                    rhs=(rhsB[:, n0:n0 + NCH]),
                    start=True, stop=True,
                )
            t = p_tmp.tile([128, QC], F32R, tag="tmp", name="lnt")
            nc.vector.tensor_mul(t, blocks[m], a_ps)
            o = out_pool.tile([128, QC], out_dtype, tag=out_tag)
            nc.vector.tensor_add(o, t, b_ps)
            outs.append(o)
        return outs

    y0 = ln_finish(acc_sum0, acc_sq0, gb0, x, p_work, "work", out_dtype=BF16)

    acc1 = p_acc.tile([1, 2 * QC], F32, tag="acc1", name="acc1")
    acc_sum1, acc_sq1 = acc1[:, 0:QC], acc1[:, QC:2 * QC]
    z = []
    for o in range(CB):
        ocols = slice(o * 128, (o + 1) * 128)
        cps = ps_log.tile([128, QC], F32, tag="log", name="cps")
        for n0 in range(0, QC, NCH):
            for ci in range(CB):
                nc.tensor.matmul(
                    out=cps[:, n0:n0 + NCH],
                    lhsT=(woT[ci][:, ocols]),
                    rhs=(y0[ci][:, n0:n0 + NCH]),
                    start=(ci == 0), stop=(ci == CB - 1),
                )
        r_t = p_tmp.tile([128, QC], F32R, tag="tmp", name="relu")
        nc.scalar.activation(r_t, cps, RELU, bias=bo_pp[:, o:o + 1], scale=1.0)
        zo = p_xz.tile([128, QC], F32R, tag="xz", name="xz")
        nc.gpsimd.tensor_add(zo, r_t, y0[o])
        z.append(zo)
        # z^2 + LN1 partial stats interleaved with the conv loop
        sq_z = p_sq.tile([128, QC], BF16, tag="sq", name="sqz")
        nc.gpsimd.tensor_mul(sq_z, zo, zo)
        emit_stats_partial(zo, sq_z, acc_sum1, acc_sq1, o == 0)

    fin = ln_finish(acc_sum1, acc_sq1, gb1, z, p_xz, "xz")
    for m in range(CB):
        nc.sync.dma_start(out=out_ap[m * 128:(m + 1) * 128, :], in_=fin[m])


def declare_inputs(nc):
    ins = {}
    for m in range(CB):
        ins[f"scrE{m}"] = nc.dram_tensor(f"scrE{m}", [QC], F32R,
                                         kind="Internal").ap()
        ins[f"scrO{m}"] = nc.dram_tensor(f"scrO{m}", [QC], F32R,
                                         kind="Internal").ap()
    ins["Qc"] = nc.dram_tensor("Qc", [D, QC], F32R, kind="ExternalInput").ap()
    ins["Kc"] = nc.dram_tensor("Kc", [D, SK], F32R, kind="ExternalInput").ap()
    for w in ("WqT", "WkT", "WvT", "WoT"):
        ins[w] = nc.dram_tensor(w, [D, D], F32R, kind="ExternalInput").ap()
    for vname in ("bq", "bk", "bv", "bo"):
        ins[vname] = nc.dram_tensor(vname, [D], F32, kind="ExternalInput").ap()
    for vname in ("g0", "b0", "g1", "b1"):
        ins[vname] = nc.dram_tensor(vname, [D], F32R, kind="ExternalInput").ap()
    ins["ones_c"] = nc.dram_tensor("ones_c", [128, H], F32R,
                                   kind="ExternalInput").ap()
    ins["ones_q"] = nc.dram_tensor("ones_q", [QC], F32R,
                                   kind="ExternalInput").ap()
    return ins


def make_in_maps(inputs):
    Q = np.ascontiguousarray(np.asarray(inputs["Q"], dtype=np.float32))
    K = np.ascontiguousarray(np.asarray(inputs["K"], dtype=np.float32))
    shared = {
        "WqT": np.ascontiguousarray(np.asarray(inputs["Wq"], np.float32).T),
        "WkT": np.ascontiguousarray(np.asarray(inputs["Wk"], np.float32).T),
        "WvT": np.ascontiguousarray(np.asarray(inputs["Wv"], np.float32).T),
        "WoT": np.ascontiguousarray(np.asarray(inputs["Wo"], np.float32).T),
        "bq": np.asarray(inputs["bq"], np.float32),
        "bk": np.asarray(inputs["bk"], np.float32),
        "bv": np.asarray(inputs["bv"], np.float32),
        "bo": np.asarray(inputs["bo"], np.float32),
        "g0": np.asarray(inputs["gamma0"], np.float32),
        "b0": np.asarray(inputs["beta0"], np.float32),
        "g1": np.asarray(inputs["gamma1"], np.float32),
        "b1": np.asarray(inputs["beta1"], np.float32),
        "ones_c": np.ones((128, H), dtype=np.float32),
        "ones_q": np.ones((QC,), dtype=np.float32),
    }
    in_maps = []
    for core in range(8):
        b, j = core // 2, core % 2
        m = dict(shared)
        m["Qc"] = np.ascontiguousarray(Q[b, :, j * QC:(j + 1) * QC])
        m["Kc"] = np.ascontiguousarray(K[b])
        in_maps.append(m)
    return in_maps


def assemble_output(res):
    out = np.empty((B, D, SQ), dtype=np.float32)
    for core in range(8):
        b, j = core // 2, core % 2
        out[b, :, j * QC:(j + 1) * QC] = res.results[core]["out"]
    return out


def build_module():
    nc = bacc.Bacc("TRN2", target_bir_lowering=False, debug=False)
    ins = declare_inputs(nc)
    out_ap = nc.dram_tensor("out", [D, QC], F32R, kind="ExternalOutput").ap()

    with tile.TileContext(nc) as tc:
        with nc.allow_low_precision(reason="f32r/bf16 tiles feed full-rate matmuls"):
            with ExitStack() as ctx:
                emit_core_kernel(ctx, tc, ins, out_ap)
    nc.compile()
    return nc


_NC_CACHE = None


def _get_nc():
    global _NC_CACHE
    if _NC_CACHE is None:
        _NC_CACHE = build_module()
    return _NC_CACHE


def kernel(Q, K, Wq, bq, Wk, bk, Wv, bv, Wo, bo, gamma0, beta0, gamma1, beta1,
           _trace=False, _trace_cores=None):
    in_maps = make_in_maps(dict(
        Q=Q, K=K, Wq=Wq, bq=bq, Wk=Wk, bk=bk, Wv=Wv, bv=bv, Wo=Wo, bo=bo,
        gamma0=gamma0, beta0=beta0, gamma1=gamma1, beta1=beta1))

    nc = _get_nc()
    res = run_bass_kernel_spmd(
        nc, in_maps, core_ids=list(range(8)),
        trace=_trace, trace_cores=_trace_cores,
    )
    out = assemble_output(res)
    if _trace:
        kernel._last_result = res
    return out
